# revision 1
# baseline (speedup 1.0000x reference)
"""Multi-head attention (B=4, S=2048, D=512, H=8) on 8 TRN2 NeuronCores.

Sharding: core c handles batch b = c//2 and query-half q = c%2 (1024 query
rows). Attention needs all keys/values of the batch, so K/V projections are
duplicated between the two cores of a batch pair; there is no cross-core
communication. Each core returns out[b, half] = [1024, 512]. Inputs are
handed to each core already transposed ([d_in, s]) — transposition is part
of the host-side sharding/marshalling, so the PE does no transposes.

Per-core dataflow (fp32 storage, float32r matmuls):
  1. q^T = matmul(lhsT=W_q, rhs=x_q^T) -> [d_out, s] (head-major partitions),
     same for k^T. v is produced in natural [s, d_out] layout
     (lhsT=x_v^T slice, rhs=W_v) and scattered into a [s, 8*65] "augmented"
     layout whose ones column per head makes the PV matmul also emit the
     softmax denominator.
  2. Flat pipeline over (head, key-block) slots: scores^T = k^T.T @ q^T in
     PSUM; P^T = exp(scores^T/8) on ACT (no max subtraction: |scores/8| < ~6);
     the PV matmul out^T[65,1024] += v_aug.T @ P^T trails DELAY slots behind
     so head boundaries never stall the ACT exp pipeline.
  3. Normalize per head: copy PSUM->SBUF (fast slot release), reciprocal of
     the denominator row, GPSIMD partition-broadcast, multiply into outT.
  4. final = outT.T @ W_out -> natural [s, 512] -> DMA out.

Engine budget per core (cost model): ACT exp ~123us is the long pole;
PE attention (~110us) overlaps the exp stream. PSUM: ppB 2 + st 2x2 + pv 2
= 8 banks in the attention phase.
"""

import numpy as np

import concourse.bass as bass
from concourse import bacc
import concourse.mybir as mybir
import concourse.tile as tile
from concourse.bass_utils import run_bass_kernel_spmd

B, S, D, H = 4, 2048, 512, 8
DH = D // H          # 64
P = 128
SQ = S // 2          # 1024 query rows per core
NCORES = 8
F32 = mybir.dt.float32
F32R = mybir.dt.float32r
EXP = mybir.ActivationFunctionType.Exp
SCALE = 1.0 / np.sqrt(DH)  # 0.125


def _r(ap):
    return ap.bitcast(F32R)


def _build_mha(tc, out_d, xqT_d, xkT_d, xvT_d, wq_d, wk_d, wv_d, wo_d):
    nc = tc.nc
    NKB = S // P       # 16 key blocks
    NQC = SQ // 512    # 2 query column chunks of 512
    VW = DH + 1        # 65: per-head v columns + ones column

    dma_rr = [0]

    def dma(out, in_):
        eng = nc.sync if dma_rr[0] % 2 == 0 else nc.scalar
        dma_rr[0] += 1
        eng.dma_start(out, in_)

    with (
        tc.tile_pool(name="consts", bufs=1) as cpool,
        tc.tile_pool(name="big", bufs=1) as bpool,
        tc.tile_pool(name="work", bufs=2) as wpool,
    ):
        # x^T chunk loader: [128 (d_in chunk c), 512 (s chunk n)] tiles,
        # split into column halves across the SP/ACT DMA queues.
        def load_xT(xT_d, c, n, pieces=1, name="xt", issuer=None):
            t = wpool.tile([P, 512], F32R, tag="xT", bufs=10, name=name)
            src = _r(xT_d[c * P : (c + 1) * P, n * 512 : (n + 1) * 512])
            w = 512 // pieces
            for pc in range(pieces):
                if issuer is None:
                    dma(t[:, pc * w : (pc + 1) * w], src[:, pc * w : (pc + 1) * w])
                else:
                    issuer.dma_start(
                        t[:, pc * w : (pc + 1) * w], src[:, pc * w : (pc + 1) * w]
                    )
            return t

        # q chunk 0 goes out before the weight DMAs so the PE can start early
        first_xq = [load_xT(xqT_d, c, 0, name="xt_first") for c in range(4)]

        # Weights, natural layout, d_in-chunked: w[:, c, :] = W[c*128:(c+1)*128, :].
        # Loaded via the (otherwise idle) SWDGE/gpsimd queues.
        wq_sb = cpool.tile([P, 4, D], F32R)
        wk_sb = cpool.tile([P, 4, D], F32R)
        wv_sb = cpool.tile([P, 4, D], F32R)
        wo_sb = cpool.tile([P, 4, D], F32R)
        for w_sb, w_d in ((wq_sb, wq_d), (wk_sb, wk_d), (wv_sb, wv_d), (wo_sb, wo_d)):
            wr = _r(w_d.rearrange("(c p) n -> p c n", p=P))
            for c in range(4):
                for pc in range(2):
                    nc.gpsimd.dma_start(
                        w_sb[:, c, pc * 256 : (pc + 1) * 256],
                        wr[:, c, pc * 256 : (pc + 1) * 256],
                    )

        # Big single-buffer tensors that live through the attention phase.
        qT = bpool.tile([P, 4, SQ], F32R)    # [d_out%128, d_out//128, s]
        kT = bpool.tile([P, 4, S], F32R)
        v_aug = bpool.tile([P, NKB, H * VW], F32R)  # [s%128, s//128, h*65+dv]
        outT = bpool.tile([P, 4, SQ], F32R)

        # Dummy exp pulls the ACT exp-table load to t=0.
        warm = cpool.tile([P, 1], F32)
        nc.scalar.activation(warm, wq_sb.bitcast(F32)[:, 0, 0:1], EXP)

        # Fill v_aug with ones; projection copies overwrite the value columns,
        # leaving a ones column per head at offset 64. (memset can't write
        # f32r, so go through tensor_scalar 0*x+1.)
        nc.vector.tensor_scalar(
            out=v_aug.rearrange("p n e -> p (n e)"),
            in0=wq_sb.bitcast(F32)[:, 0, 0:1].broadcast_to([P, NKB * H * VW]),
            scalar1=0.0,
            scalar2=1.0,
            op0=mybir.AluOpType.mult,
            op1=mybir.AluOpType.add,
        )

        # ---------------- q/k projections (phase A) ----------------
        ppB_cm = tc.tile_pool(name="ps_ppB", bufs=2, space="PSUM")
        ps_ppB = ppB_cm.__enter__()

        def project_v_chunk(n, preloaded=None):
            # generator: yields between sections so emission can spread
            # across early attention slots
            if preloaded is not None:
                xTs = preloaded
            else:
                xTs = [load_xT(xvT_d, c, n, name="xt_v", issuer=nc.sync) for c in range(4)]
            yield
            for sb in range(4):
                pp = ps_ppB.tile([P, 512], F32, tag="ppB", name="pp_v")
                for c in range(4):
                    nc.tensor.matmul(
                        pp,
                        xTs[c][:, sb * P : (sb + 1) * P],
                        wv_sb[:, c, :],
                        start=(c == 0),
                        stop=(c == 3),
                    )
                nc.vector.tensor_copy(
                    v_aug.rearrange("p n (h e) -> p n h e", e=VW)[
                        :, n * 4 + sb, :, 0:DH
                    ],
                    pp.rearrange("p (h d) -> p h d", d=DH),
                )
                yield

        first_xv = None
        with tc.tile_pool(name="ps_ppA", bufs=6, space="PSUM") as ps_ppA:
            def project_T(xT_d, w_sb, dst, s_len, preloaded=None, hook=None):
                for n in range(s_len // 512):
                    if n == 0 and preloaded is not None:
                        xTs = preloaded
                    else:
                        xTs = [load_xT(xT_d, c, n) for c in range(4)]
                    for m in range(4):
                        pp = ps_ppA.tile([P, 512], F32, tag="ppA", name="pp_t")
                        for c in range(4):
                            nc.tensor.matmul(
                                pp,
                                w_sb[:, c, m * P : (m + 1) * P],
                                xTs[c],
                                start=(c == 0),
                                stop=(c == 3),
                            )
                        nc.vector.tensor_copy(dst[:, m, n * 512 : (n + 1) * 512], pp)
                    if hook is not None:
                        hook(n)

            project_T(xqT_d, wq_sb, qT, SQ, preloaded=first_xq)

            def k_hook(n):
                nonlocal first_xv
                if n == 1:
                    # v chunk-0 loads issue here so data is resident when the
                    # v matmuls run right after the k projection
                    first_xv = [load_xT(xvT_d, c, 0, name="xt_v0") for c in range(4)]

            project_T(xkT_d, wk_sb, kT, S, hook=k_hook)

            vg0 = project_v_chunk(0, preloaded=first_xv)
            next(vg0, None)   # skip the (empty) load section
            for _ in vg0:
                pass

        # ---------------- attention (phase B) ----------------
        with (
            tc.tile_pool(name="ps_st", bufs=2, space="PSUM") as ps_st,
            tc.tile_pool(name="ps_pv", bufs=1, space="PSUM") as ps_pv,
        ):

            # Flat pipeline over (head, key-block) slots; PV trails by DELAY.
            DELAY = 4
            seq = [(h, blk) for h in range(H) for blk in range(NKB)]
            vgens = [project_v_chunk(n) for n in range(1, 4)]
            fifo = []
            pv_tiles = {}

            def emit_pv(h, blk, pT):
                po = (h % 2) * DH
                mc = h // 2
                if blk == 0:
                    pv_tiles[h] = ps_pv.tile([P, SQ], F32, tag="pv", name="pv")
                pv = pv_tiles[h]
                for nq in range(NQC):
                    nc.tensor.matmul(
                        pv[0 : VW, nq * 512 : (nq + 1) * 512],
                        v_aug[:, blk, h * VW : (h + 1) * VW],
                        pT[:, nq * 512 : (nq + 1) * 512],
                        start=(blk == 0),
                        stop=(blk == NKB - 1),
                    )
                if blk == NKB - 1:
                    if h < H - 1:
                        # single fast copy releases the PSUM slot; the
                        # normalization runs off the critical path from SBUF
                        pvc = wpool.tile([VW, SQ], F32, tag="pvc", bufs=2)
                        nc.vector.tensor_copy(pvc, pv[0:VW, :])
                        src_ap = pvc
                    else:
                        # last head: no successor needs the slot, normalize
                        # straight from PSUM (shorter critical chain)
                        src_ap = pv
                    recip = wpool.tile([1, SQ], F32, tag="recip", bufs=2)
                    nc.vector.reciprocal(recip, src_ap[DH : DH + 1, :])
                    bcast = wpool.tile([DH, SQ], F32, tag="bcast", bufs=2)
                    nc.gpsimd.partition_broadcast(bcast, recip)
                    nc.vector.tensor_mul(
                        outT[po : po + DH, mc, :], src_ap[0:DH, :], bcast
                    )
                    del pv_tiles[h]

            for h, blk in seq:
                for _ in range(1):
                    if vgens:
                        if next(vgens[0], "done") == "done":
                            vgens.pop(0)
                po = (h % 2) * DH
                mc = h // 2
                kT_h = kT[po : po + DH, mc, :]
                qT_h = qT[po : po + DH, mc, :]
                st = ps_st.tile([P, SQ], F32, tag="st")
                for nq in range(NQC):
                    nc.tensor.matmul(
                        st[:, nq * 512 : (nq + 1) * 512],
                        kT_h[:, blk * P : (blk + 1) * P],
                        qT_h[:, nq * 512 : (nq + 1) * 512],
                        start=True,
                        stop=True,
                    )
                pT = wpool.tile([P, SQ], F32R, tag="pT", bufs=DELAY + 2)
                nc.scalar.activation(pT, st, EXP, scale=float(SCALE))
                fifo.append((h, blk, pT))
                if len(fifo) > DELAY:
                    emit_pv(*fifo.pop(0))
            while fifo:
                emit_pv(*fifo.pop(0))

        ppB_cm.__exit__(None, None, None)

        # ---------------- output projection ----------------
        with tc.tile_pool(name="ps_f", bufs=4, space="PSUM") as ps_f:
            for nb in range(SQ // P):
                pf = ps_f.tile([P, D], F32, tag="pf")
                for c in range(4):
                    nc.tensor.matmul(
                        pf,
                        outT[:, c, nb * P : (nb + 1) * P],
                        wo_sb[:, c, :],
                        start=(c == 0),
                        stop=(c == 3),
                    )
                ob = wpool.tile([P, D], F32, tag="ob", bufs=4)
                nc.vector.tensor_copy(ob, pf)
                for pc in range(2):
                    nc.sync.dma_start(
                        out_d[nb * P : (nb + 1) * P, pc * 256 : (pc + 1) * 256],
                        ob[:, pc * 256 : (pc + 1) * 256],
                    )


_CACHED_NC = None


def _get_nc():
    global _CACHED_NC
    if _CACHED_NC is not None:
        return _CACHED_NC
    nc = bacc.Bacc("TRN2", target_bir_lowering=False, debug=False)
    xqT = nc.dram_tensor("xqT", [D, SQ], F32, kind="ExternalInput").ap()
    xkT = nc.dram_tensor("xkT", [D, S], F32, kind="ExternalInput").ap()
    xvT = nc.dram_tensor("xvT", [D, S], F32, kind="ExternalInput").ap()
    wq = nc.dram_tensor("wq", [D, D], F32, kind="ExternalInput").ap()
    wk = nc.dram_tensor("wk", [D, D], F32, kind="ExternalInput").ap()
    wv = nc.dram_tensor("wv", [D, D], F32, kind="ExternalInput").ap()
    wo = nc.dram_tensor("wo", [D, D], F32, kind="ExternalInput").ap()
    out = nc.dram_tensor("out", [SQ, D], F32, kind="ExternalOutput").ap()
    with tile.TileContext(nc) as tc:
        _build_mha(tc, out, xqT, xkT, xvT, wq, wk, wv, wo)
    nc.compile()
    _CACHED_NC = nc
    return nc


def _run(in_query, in_key, in_value, W_q, W_k, W_v, W_out, **run_kwargs):
    f = lambda a: np.ascontiguousarray(np.asarray(a), dtype=np.float32)
    in_query, in_key, in_value = f(in_query), f(in_key), f(in_value)
    W_q, W_k, W_v, W_out = f(W_q), f(W_k), f(W_v), f(W_out)
    xkT = [f(in_key[b].T) for b in range(B)]
    xvT = [f(in_value[b].T) for b in range(B)]
    in_maps = []
    for c in range(NCORES):
        b, half = c // 2, c % 2
        in_maps.append(
            {
                "xqT": f(in_query[b, half * SQ : (half + 1) * SQ, :].T),
                "xkT": xkT[b],
                "xvT": xvT[b],
                "wq": W_q,
                "wk": W_k,
                "wv": W_v,
                "wo": W_out,
            }
        )
    res = run_bass_kernel_spmd(_get_nc(), in_maps, list(range(NCORES)), **run_kwargs)
    out = np.empty((B, S, D), np.float32)
    for c in range(NCORES):
        b, half = c // 2, c % 2
        out[b, half * SQ : (half + 1) * SQ, :] = res.results[c]["out"]
    return out, res


def kernel(in_query, in_key, in_value, W_q, W_k, W_v, W_out):
    out, _ = _run(in_query, in_key, in_value, W_q, W_k, W_v, W_out)
    return out

